# revision 10
# baseline (speedup 1.0000x reference)
"""Trainium2 Bass kernel for nn_CONVClassifier (embedding -> pair-conv -> maxpool
-> sigmoid -> 2-layer classifier -> log_softmax).

Sharding: data-parallel over batch. 64 sequences / 8 cores = 8 sequences per core.
Weights replicated; each core gets a host-compacted per-core embedding table
(<=2048 unique tokens per core) so the device gather uses int16/int32 row ids.

v7 structure (hybrid gather):
  - Pairs 0-1 (positions 0..1023): 8 library-free indirect DMAs (128 fp16
    rows each) + PE transposes + DVE copy-converts into clean-chunk fp8
    segments. This path needs no GPSIMD library, so it runs during the
    ~10us mlp ucode-load window and the conv starts at ~16us.
  - Pairs 2-3: fp8 transposing dma_gather (TIE, 16-bit granularity, so
    partition p holds the byte pair e=256c+2p(+1) - a DoubleRow plane pair
    consumed via a stride-(1,2) interleaved AP). Q7 order: 8 indirects ->
    lib load -> 2 gathers; their data hides behind pairs 0-1's compute.
  - Conv matmuls in fp8e4 perf_mode=DoubleRow (K=256, N=512/511; the +1
    boundary column is maxpool-excluded so segments are exactly 512 wide).
    Two host weight layouts match the two segment pairings.
  - Emb and Wc pre-scaled by 512; the 2^-18 descale folds into the Exp
    activation's scale. Exp is the ONLY table function: sent = 1/(1+exp(-t))
    via Exp + DVE reciprocal; W2@W1 folded on host into 8 tiny f16
    accumulating matmuls hidden in-stream; the 2-class log_softmax tail
    -ln(1+exp(delta)) uses a bit-hack log2 + one exp-based Newton step.
"""

import numpy as np
from contextlib import ExitStack

import ml_dtypes

import concourse.bass as bass
import concourse.tile as tile
from concourse import bacc, mybir, library_config
from concourse.bass_utils import run_bass_kernel_spmd
from concourse.tile import add_dep_helper

# Problem shapes (hardcoded per harness contract).
V, E, S, NCLASS = 50000, 512, 1024, 2
B, L = 64, 256
NCORES = 8
BLOC = B // NCORES          # 8 sequences per core
POS = BLOC * L              # 2048 positions per core
PAIRS = BLOC // 2           # 4 sequence-pairs (N=512 per matmul group)
SEG = 512                   # positions per segment (boundary col is excluded)
SC = S // 128               # 8 output-channel chunks
KK = 4                      # DoubleRow k-superchunks (K=256 each, f = 2E)
EC = E // 128               # 4 embedding chunks
UT = 2048                   # compacted per-core table rows (>= unique tokens)
SCALE_BITS = 18             # emb*512 * Wc*512 -> psum scaled by 2^18
NIND = 8                    # indirect-gathered 128-row tiles (pairs 0-1)

# cb16 blob layout (int16 units)
CB_IND = 0                  # 2 segs (pairs 2,3) * SEG/16 = 64 idx cols
CB_I32 = 64                 # 8 int32 (16 i16) indirect row-id columns
CB_WD = 80                  # 16 cols weffd f16 bits
CB_IDT = 96                 # 128 cols identity f16 bits
CB16_W = 224

F32 = mybir.dt.float32
F16 = mybir.dt.float16
F8 = mybir.dt.float8e4
I16 = mybir.dt.int16
I32 = mybir.dt.int32
AF = mybir.ActivationFunctionType
ALU = mybir.AluOpType
DR = mybir.MatmulPerfMode.DoubleRow

# fast-log constants: ln(v) ~= bits(v)*C1 - C2, one Newton step via exp
C1 = float(np.log(2.0) / (1 << 23))
C2P1 = float((127.0 - 0.043) * np.log(2.0) + 1.0)   # c2 + 1 (folds the -1)

_CACHE = {}


def build_program():
    nc = bacc.Bacc("TRN2", target_bir_lowering=False, debug=False,
                   num_devices=NCORES, enable_partition_id=False)

    table16 = nc.dram_tensor("table16", [UT, E], F16, kind="ExternalInput")
    table8 = nc.dram_tensor("table8", [UT, E], F8, kind="ExternalInput")
    cb16 = nc.dram_tensor("cb16", [128, CB16_W], I16, kind="ExternalInput")
    cb32 = nc.dram_tensor("cb32", [128, 26], F32, kind="ExternalInput")
    # wct8[:, 0] pairs planes 128 apart (clean chunk layout, pairs 0-1);
    # wct8[:, 1] pairs planes (2p, 2p+1) (gather interleave, pairs 2-3).
    wct8 = nc.dram_tensor("wct8", [128, 2, SC, KK, 2, 128], F8,
                          kind="ExternalInput")
    out_d = nc.dram_tensor("out", [NCLASS, BLOC], F32, kind="ExternalOutput")

    with tile.TileContext(nc) as tc, ExitStack() as ctx:
        const = ctx.enter_context(tc.tile_pool(name="const", bufs=1))
        trp = ctx.enter_context(
            tc.tile_pool(name="trp", bufs=2, space=bass.MemorySpace.PSUM))
        warmp = ctx.enter_context(
            tc.tile_pool(name="warmp", bufs=1, space=bass.MemorySpace.PSUM))
        mmp = ctx.enter_context(
            tc.tile_pool(name="mmp", bufs=4, space=bass.MemorySpace.PSUM))
        tailp = ctx.enter_context(
            tc.tile_pool(name="tailp", bufs=1, space=bass.MemorySpace.PSUM))

        # --- const DMAs: indices on scalar (land ~8.3us), weights on sync
        # with the pair0/1 layout first. ---
        cb16_sb = const.tile([128, CB16_W], I16)
        nc.scalar.dma_start(cb16_sb[:], cb16[:])
        cb32_sb = const.tile([128, 26], F32)
        nc.scalar.dma_start(cb32_sb[:], cb32[:])
        wct8_sb = const.tile([128, 2, SC, KK, 2, 128], F8)
        nc.sync.dma_start(wct8_sb[:, 0], wct8[:, 0])
        nc.sync.dma_start(wct8_sb[:, 1], wct8[:, 1])

        ind32_v = cb16_sb[:, CB_I32:CB_I32 + 16].bitcast(I32)   # [128, 8]
        weffd_v = cb16_sb[:, CB_WD:CB_WD + 16].bitcast(F16)     # [128, 16]
        ident_v = cb16_sb[:, CB_IDT:CB_IDT + 128].bitcast(F16)  # [128, 128]

        # --- PE warmup weights + ACT table preload, independent of DMAs ---
        warm_w = const.tile([128, 2 * L], F16)
        nc.vector.memset(warm_w[:], 0.0)
        warm_w32 = const.tile([128, 2 * L], F32)
        nc.vector.memset(warm_w32[:], 0.0)
        dume = const.tile([1, 2], F32)
        nc.scalar.activation(out=dume[:], in_=warm_w[0:1, 0:2], func=AF.Exp)

        warm = warmp.tile([128, 2 * L], F32, tag="warm")

        def warmup(n, f32=False):
            w = warm_w32 if f32 else warm_w
            for _ in range(n):
                nc.tensor.matmul(warm[:], lhsT=w[:, 0:128],
                                 rhs=w[:], start=True, stop=True)

        # --- Q7: 8 library-free indirect gathers (pairs 0-1), then the mlp
        # lib load (its ~10us ucode DMA overlaps pair-0 compute), then the
        # two fp8 transposing gathers for pairs 2-3. ---
        q7_last = None

        def q7_chain(gi):
            nonlocal q7_last
            if q7_last is not None:
                add_dep_helper(gi.ins, q7_last.ins, sync=False,
                               reason="q7 serial order")
            q7_last = gi

        raws = []
        for t in range(NIND):
            raw = const.tile([128, E], F16, tag=f"raw{t}", name=f"raw{t}")
            gi = nc.gpsimd.indirect_dma_start(
                out=raw[:], out_offset=None, in_=table16[:],
                in_offset=bass.IndirectOffsetOnAxis(
                    ap=ind32_v[:, t:t + 1], axis=0))
            q7_chain(gi)
            raws.append(raw)

        q7_chain(nc.gpsimd.load_library(library_config.mlp))

        # pairs 0-1: clean-chunk fp8 segments filled by PE transposes
        segA = [const.tile([128, EC, SEG], F8, tag=f"segA{p}",
                           name=f"segA{p}") for p in range(2)]
        # pairs 2-3: interleaved fp8 segments straight from dma_gather
        segB = []
        for p in range(2, PAIRS):
            seg = const.tile([128, EC, SEG], F8, tag=f"segB{p}",
                             name=f"segB{p}")
            gi = nc.gpsimd.dma_gather(
                out_ap=seg[:], in_ap=table8[:],
                idxs_ap=cb16_sb[:, CB_IND + (p - 2) * (SEG // 16):
                                CB_IND + (p - 1) * (SEG // 16)],
                num_idxs=SEG, num_idxs_reg=SEG, elem_size=E, transpose=True)
            q7_chain(gi)
            segB.append(seg[:].rearrange("q c n -> q (c n)").rearrange(
                "q (c i b) -> q c b i", c=2, i=SEG, b=2))

        def transpose_tile(t):
            # raw tile t (positions 128t..128t+127) -> segA[t//4] cols
            tr = trp.tile([128, EC, 128], F16, tag="tr")
            for cc in range(EC):
                nc.tensor.transpose(out=tr[:, cc, :],
                                    in_=raws[t][:, cc * 128:(cc + 1) * 128],
                                    identity=ident_v)
            nc.vector.tensor_copy(
                out=segA[t // 4][:, :, (t % 4) * 128:(t % 4 + 1) * 128],
                in_=tr[:])

        warmup(2, f32=True)
        for t in range(4):
            transpose_tile(t)
            warmup(1)

        sent_max = [const.tile([128, BLOC], F32, tag=f"smax{sc}",
                               name=f"smax{sc}") for sc in range(SC)]
        sent_sig = [const.tile([128, BLOC], F16, tag=f"ssig{sc}",
                               name=f"ssig{sc}") for sc in range(SC)]
        e_t = [const.tile([128, BLOC], F32, tag=f"et{sc}",
                          name=f"et{sc}") for sc in range(SC)]
        z_ps = tailp.tile([NCLASS, BLOC], F32, tag="zps")

        def finish_sc(p, sc, ps):
            nc.vector.tensor_reduce(
                out=sent_max[sc][:, 2 * p:2 * p + 2],
                in_=ps[:].rearrange("q (h l) -> q h l", h=2)[:, :, 0:L - 1],
                axis=mybir.AxisListType.X, op=ALU.max)
            if p == PAIRS - 1:
                # sent = 1/(1+exp(-(max/2^18 + bc))): Exp on ACT (descale +
                # neg-bias folded in), then 1+x and reciprocal on DVE.
                nc.scalar.activation(out=e_t[sc][:], in_=sent_max[sc][:],
                                     func=AF.Exp,
                                     scale=-float(2.0 ** -SCALE_BITS),
                                     bias=cb32_sb[:, 16 + sc:17 + sc])
                nc.vector.tensor_scalar_add(e_t[sc][:], e_t[sc][:], 1.0)
                with nc.allow_low_precision("sent f16: 5e-4 rel is fine"):
                    nc.vector.reciprocal(sent_sig[sc][:], e_t[sc][:])
                # z[c,b] += (Weff[1-c]-Weff[c])[sc-chunk] . sent[sc-chunk]
                nc.tensor.matmul(z_ps[:], lhsT=weffd_v[:, 2 * sc:2 * sc + 2],
                                 rhs=sent_sig[sc][:],
                                 start=(sc == 0), stop=(sc == SC - 1))

        # conv: out[s,pos] accumulated over f=1024 as 4 DoubleRow K=256 MMs.
        # kk<2 reads the left token (shift 0), kk>=2 the right (+1 shift,
        # N=511: the boundary column is excluded by the maxpool).
        def do_pair(p):
            for sc in range(SC):
                ps = mmp.tile([128, 2 * L], F32, tag="mm")
                for kk in range(KK):
                    off = kk // 2
                    nw = 2 * L - off
                    if p < 2:
                        c0 = (kk % 2) * 2
                        rhs = segA[p][:, c0:c0 + 2, off:off + nw]
                        lhsT = wct8_sb[:, 0, sc, kk]
                    else:
                        rhs = segB[p - 2][:, kk % 2, :, off:off + nw]
                        lhsT = wct8_sb[:, 1, sc, kk]
                    nc.tensor.matmul(ps[:, 0:nw], lhsT=lhsT, rhs=rhs,
                                     start=(kk == 0), stop=(kk == KK - 1),
                                     perf_mode=DR)
                finish_sc(p, sc, ps)

        do_pair(0)
        for t in range(4, NIND):
            transpose_tile(t)
        do_pair(1)
        do_pair(2)
        do_pair(3)

        # tail: out[c,b] = -ln(v), v = 1+exp(delta_c) (delta = logit diff).
        # ln via bit-hack log2 + one Newton step (exp only, no Ln table):
        #   y0m = bits(v)*C1 - C2P1          (= ln(v) - 1, +-0.03)
        #   out = -y0m - v*exp(-1-y0m)       (abs err < 5e-4)
        e2 = const.tile([NCLASS, BLOC], F32)
        nc.scalar.activation(out=e2[:], in_=z_ps[:], func=AF.Exp,
                             bias=cb32_sb[0:NCLASS, 24:25])
        v_t = const.tile([NCLASS, BLOC], F32)
        nc.vector.tensor_scalar_add(v_t[:], e2[:], 1.0)
        iv = const.tile([NCLASS, BLOC], F32)
        nc.vector.tensor_copy(out=iv[:], in_=v_t[:].bitcast(I32))
        y0m = const.tile([NCLASS, BLOC], F32)
        nc.vector.tensor_scalar(y0m[:], iv[:], C1, -C2P1,
                                op0=ALU.mult, op1=ALU.add)
        em = const.tile([NCLASS, BLOC], F32)
        nc.scalar.activation(out=em[:], in_=y0m[:], func=AF.Exp,
                             scale=-1.0, bias=cb32_sb[0:NCLASS, 25:26])
        q_t = const.tile([NCLASS, BLOC], F32)
        nc.vector.scalar_tensor_tensor(q_t[:], v_t[:], -1.0, em[:],
                                       op0=ALU.mult, op1=ALU.mult)
        out_sb = const.tile([NCLASS, BLOC], F32)
        nc.vector.scalar_tensor_tensor(out_sb[:], y0m[:], -1.0, q_t[:],
                                       op0=ALU.mult, op1=ALU.add)
        nc.sync.dma_start(out_d[:], out_sb[:])

    nc.compile()
    return nc


def _get_program():
    if "nc" not in _CACHE:
        _CACHE["nc"] = build_program()
    return _CACHE["nc"]


def prepare_in_maps(inputs):
    inp = {k: np.asarray(v) for k, v in inputs.items()}
    idx = inp["inputs"].astype(np.int64)                       # [64, 256]
    emb = np.asarray(inp["emb_table"], dtype=np.float32)       # [V, E]
    Wc = np.asarray(inp["Wc"], dtype=np.float32)               # [S, 2E]
    bc = np.asarray(inp["bc"], dtype=np.float32)
    W1 = np.asarray(inp["W1"], dtype=np.float32)               # [50, S]
    b1 = np.asarray(inp["b1"], dtype=np.float32)
    W2 = np.asarray(inp["W2"], dtype=np.float32)               # [2, 50]
    b2 = np.asarray(inp["b2"], dtype=np.float32)

    embs = emb * 512.0
    table16_full = embs.astype(np.float16)
    table8_full = np.clip(embs, -240.0, 240.0).astype(ml_dtypes.float8_e4m3fn)

    wcs = np.clip(Wc * 512.0, -240.0, 240.0)
    # layout A (clean chunks): f = (kk//2)*512 + ((kk%2)*2 + i)*128 + p
    wa = wcs.reshape(SC, 128, 2, 2, 2, 128)      # [sc, m, lr, kkh, i, p]
    wa = wa.transpose(5, 0, 2, 3, 4, 1).reshape(128, SC, KK, 2, 128)
    # layout B (gather interleave): f = (kk//2)*512 + (kk%2)*256 + 2p + i
    wb = wcs.reshape(SC, 128, 2, 2, 128, 2)      # [sc, m, lr, kkh, p, i]
    wb = wb.transpose(4, 0, 2, 3, 5, 1).reshape(128, SC, KK, 2, 128)
    wct8 = np.ascontiguousarray(
        np.stack([wa, wb], axis=1)).astype(ml_dtypes.float8_e4m3fn)

    # folded classifier: logits = Weff@sent + beff (W1->W2 chain is linear)
    Weff = W2 @ W1                                             # [2, S]
    beff = W2 @ b1 + b2                                        # [2]
    cb32 = np.zeros((128, 26), dtype=np.float32)
    wd = np.stack([Weff[1] - Weff[0], Weff[0] - Weff[1]], axis=-1)  # [S, 2]
    wd16 = wd.reshape(SC, 128, 2).transpose(1, 0, 2).reshape(128, 16)
    cb32[:, 16:24] = -bc.reshape(SC, 128).T
    cb32[0, 24] = beff[1] - beff[0]
    cb32[1, 24] = beff[0] - beff[1]
    cb32[0:2, 25] = -1.0

    ident16 = np.eye(128, dtype=np.float16).view(np.int16)     # [128, 128]

    in_maps = []
    for c in range(NCORES):
        flat = idx[c * BLOC:(c + 1) * BLOC].reshape(-1)        # [2048]
        # Compact the table to this core's unique tokens so ids fit int16.
        uniq, inv = np.unique(flat, return_inverse=True)
        assert len(uniq) <= UT
        table_c16 = np.zeros((UT, E), dtype=np.float16)
        table_c16[:len(uniq)] = table16_full[uniq]
        table_c8 = np.zeros((UT, E), dtype=ml_dtypes.float8_e4m3fn)
        table_c8[:len(uniq)] = table8_full[uniq]
        inv16 = inv.astype(np.int16)                           # [2048]

        cb16 = np.zeros((128, CB16_W), dtype=np.int16)
        # gather idx blocks for pairs 2-3
        for p in range(2, PAIRS):
            segidx = inv16[512 * p:512 * (p + 1)]
            blk = segidx.reshape(SEG // 16, 16).T              # [16, 32]
            cb16[:, CB_IND + (p - 2) * (SEG // 16):
                 CB_IND + (p - 1) * (SEG // 16)] = np.tile(blk, (8, 1))
        # indirect row ids for pairs 0-1 (tile t = positions 128t..128t+127)
        ind32 = np.ascontiguousarray(
            inv[:NIND * 128].astype(np.int32).reshape(NIND, 128).T)
        cb16[:, CB_I32:CB_I32 + 16] = ind32.view(np.int16)
        cb16[:, CB_WD:CB_WD + 16] = wd16.astype(np.float16).view(np.int16)
        cb16[:, CB_IDT:CB_IDT + 128] = ident16

        in_maps.append({"table16": table_c16, "table8": table_c8,
                        "cb16": cb16, "cb32": cb32, "wct8": wct8})
    return in_maps


def run(inputs, trace=False):
    nc = _get_program()
    in_maps = prepare_in_maps(inputs)
    res = run_bass_kernel_spmd(nc, in_maps, list(range(NCORES)), trace=trace)
    out = np.concatenate(
        [res.results[c]["out"].T for c in range(NCORES)], axis=0)
    return out.astype(np.float32), res


def kernel(**inputs) -> np.ndarray:
    out, _ = run(inputs, trace=False)
    return out
